# revision 30
# baseline (speedup 1.0000x reference)
"""Multi-head self-attention Trainium2 kernel (nn_MultiHeadSA).

Sharding: data-parallel over the batch dim N across 8 NeuronCores
(one batch element per core). Each core computes its full [D, P] output,
the host just stacks the per-core results.

Math (per batch n, head h), restructured for the PE-friendly [k, q]
attention layout with softmax along the PSUM partition (key) axis:

  logits[k,q] = (Wk_h x + bk)^T (Wq_h x + bq) / sqrt(D) + pos[h,k,q]
              = x^T Gh x  +  term_k[k]  +  (terms const in k -> drop
                under softmax)  + pos[h,k,q]
     Gh  = Wk_h^T Wq_h / sqrt(D)      (host-precomputed)
     term_k = x^T u_h,  u_h = Wk_h^T bq_h / sqrt(D)

  y    = Gh x                         (PE, lhsT = Gh^T)
  attn = x^T y + pos                  (PE; pos injected by identity-matmul
                                       accumulation into the same PSUM bank)
  E    = exp(attn + term_k)           (ScalarE, term_k as per-partition bias)
  s[q] = 1^T E                        (PE ones-matmul; softmax denominator)
  out_h = (Wv_h x) E * (1/s)          (PE + DVE; bv folded into bo')
  fin  = sum_h Wo_h out_h + bo'       (PE; bo' = bo + Wo bv, per-part bias)

All heavy matmuls run as float32r (TF32 datapath, full PE rate at
free-dim >= 256). The final Wo projection per (head, q-block) is
software-pipelined one attention-block behind so the PE's in-order
stream never head-of-line blocks on the softmax normalization chain.
"""

import numpy as np

try:
    import concourse.bass as bass
except ImportError:  # pragma: no cover
    import sys

    sys.path.insert(0, "/opt/trn_rl_repo")
    import concourse.bass as bass

from contextlib import ExitStack

import concourse.bacc as bacc
import concourse.mybir as mybir
import concourse.tile as tile

F32 = mybir.dt.float32
F32R = mybir.dt.float32r
FP16 = mybir.dt.float16

N, D, P, H = 8, 256, 1024, 8
QW = 512  # q-block width (PSUM bank / fp32 moving-operand limit)


def build_nc(h_num=H, d=D, p=P, reps=1):
    assert d % 128 == 0 and p % QW == 0 and p % 128 == 0
    IC = d // 128  # input-dim (contraction) chunks
    KC = p // 128  # key chunks
    QB = p // QW  # query blocks
    OC = d // 128  # output-dim chunks

    nc = bacc.Bacc(None, target_bir_lowering=False)

    x_d = nc.dram_tensor("x", [d, p], F32R, kind="ExternalInput")
    pos_d = nc.dram_tensor("pos", [h_num, p, p], F32R, kind="ExternalInput")
    # gt[h] = (Wk_h^T Wq_h / sqrt(D))^T = Wq_h^T Wk_h / sqrt(D): [i', i]
    gt_d = nc.dram_tensor("gt", [h_num, d, d], F32R, kind="ExternalInput")
    u_d = nc.dram_tensor("u", [d, h_num], F32, kind="ExternalInput")
    wv_d = nc.dram_tensor("wv", [d, h_num * d], F32R, kind="ExternalInput")  # Wv^T
    wo_d = nc.dram_tensor("wo", [h_num * d, d], F32R, kind="ExternalInput")  # Wo^T
    bo_d = nc.dram_tensor("bo", [d], F32, kind="ExternalInput")  # bo + Wo bv
    id_d = nc.dram_tensor("ident", [128, 128], F32R, kind="ExternalInput")
    on_d = nc.dram_tensor("onescol", [128, 1], F32R, kind="ExternalInput")
    out_d = nc.dram_tensor("out", [d, p], F32, kind="ExternalOutput")

    with tile.TileContext(nc) as tc, ExitStack() as ctx:
        const = ctx.enter_context(tc.tile_pool(name="const", bufs=1))
        pos_pool = ctx.enter_context(tc.tile_pool(name="pos", bufs=8))
        hbufs = ctx.enter_context(tc.tile_pool(name="hbufs", bufs=2))
        ohp = ctx.enter_context(tc.tile_pool(name="ohp", bufs=6))
        ebufs = ctx.enter_context(tc.tile_pool(name="ebufs", bufs=2))
        finp = ctx.enter_context(tc.tile_pool(name="finp", bufs=1))

        ps_at = ctx.enter_context(tc.tile_pool(name="ps_at", bufs=2, space="PSUM"))
        ps_s = ctx.enter_context(tc.tile_pool(name="ps_s", bufs=1, space="PSUM"))
        ps_av = ctx.enter_context(tc.tile_pool(name="ps_av", bufs=3, space="PSUM"))
        ps_sc = ctx.enter_context(tc.tile_pool(name="ps_sc", bufs=2, space="PSUM"))

        # ---- constants (head-0 slices first so compute starts early) ----
        x_sb = const.tile([128, IC, p], F32R)
        x_r = x_d.rearrange("(c r) p -> r c p", r=128)
        for c in range(IC):
            nc.sync.dma_start(
                out=x_sb[:, c, bass.ts(0, QW)], in_=x_r[:, c, bass.ts(0, QW)]
            )

        gt_sb = const.tile([128, IC, h_num, d], F32R)
        gt_r = gt_d.rearrange("h (c r) i -> r c h i", r=128)
        wv_sb = const.tile([128, IC, h_num * d], F32R)
        wv_r = wv_d.rearrange("(c r) o -> r c o", r=128)
        wo_sb = const.tile([128, IC * h_num, d], F32R)
        wo_r = wo_d.rearrange("(c r) o -> r c o", r=128)

        def load_head_weights(hh, with_wo):
            for c in range(IC):
                nc.sync.dma_start(out=gt_sb[:, c, hh, :], in_=gt_r[:, c, hh, :])
            for c in range(IC):
                nc.sync.dma_start(
                    out=wv_sb[:, c, bass.ds(hh * d, d)],
                    in_=wv_r[:, c, bass.ds(hh * d, d)],
                )
                if with_wo:
                    nc.sync.dma_start(
                        out=wo_sb[:, IC * hh + c, :], in_=wo_r[:, IC * hh + c, :]
                    )

        u_sb = const.tile([128, IC, h_num], F32)
        nc.sync.dma_start(out=u_sb, in_=u_d.rearrange("(c r) h -> r c h", r=128))

        load_head_weights(0, with_wo=False)

        # identity (bf16, for the pos inject) + f32r ones column
        ident = const.tile([128, 128], F32R, name="ident")
        nc.sync.dma_start(out=ident, in_=id_d[:, :])
        ones_col = const.tile([128, 1], F32R, name="ones_col")
        nc.sync.dma_start(out=ones_col, in_=on_d[:, :])
        ones_row = const.tile([1, 128], F32)
        nc.vector.memset(ones_row, 1.0)

        for qh in range(1, p // QW):
            for c in range(IC):
                nc.sync.dma_start(
                    out=x_sb[:, c, bass.ts(qh, QW)],
                    in_=x_r[:, c, bass.ts(qh, QW)],
                )

        bo_sb = const.tile([128, OC], F32)
        nc.sync.dma_start(out=bo_sb, in_=bo_d.rearrange("(c r) -> r c", r=128))

        fin_sb = finp.tile([128, OC, p], F32)

        # deferred PE-tail state per q-block: [(h, oh_sb), ...]
        pending = {qb: [] for qb in range(QB)}

        def emit_proj(pqb, group, last_head):
            first = group[0][0] == 0
            for oc in range(OC):
                pj_ps = ps_sc.tile([128, QW], F32, tag="sc", name="pj")
                nmm = len(group) * IC
                i = 0
                for ph, poh in group:
                    for c in range(IC):
                        nc.tensor.matmul(
                            pj_ps,
                            wo_sb[:, IC * ph + c, bass.ts(oc, 128)],
                            poh[:, c, :],
                            start=(i == 0),
                            stop=(i == nmm - 1),
                        )
                        i += 1
                dst = fin_sb[:, oc, bass.ds(pqb * QW, QW)]
                if first:
                    nc.vector.tensor_scalar_add(
                        out=dst, in0=pj_ps, scalar1=bo_sb[:, oc : oc + 1]
                    )
                else:
                    nc.vector.tensor_add(dst, dst, pj_ps)
                if last_head:
                    nc.sync.dma_start(
                        out=out_d[bass.ts(oc, 128), bass.ds(pqb * QW, QW)],
                        in_=dst,
                    )

        for _rep, h in [(r0, h0) for r0 in range(reps) for h0 in range(h_num)]:
            if _rep == 0:
                if h + 1 < h_num:
                    load_head_weights(h + 1, with_wo=False)
                for c in range(IC):
                    nc.sync.dma_start(
                        out=wo_sb[:, IC * h + c, :], in_=wo_r[:, IC * h + c, :]
                    )
            # ---- term_k[k] = x^T u_h  (per-partition exp bias) ----
            tk_ps = ps_at.tile([128, KC], F32, tag="at_ps", name="tk")
            for kc in range(KC):
                for c in range(IC):
                    nc.tensor.matmul(
                        tk_ps[:, kc : kc + 1],
                        x_sb[:, c, bass.ts(kc, 128)].bitcast(F32),
                        u_sb[:, c, h : h + 1],
                        start=(c == 0),
                        stop=(c == IC - 1),
                    )
            tk_sb = hbufs.tile([128, KC], F32)
            nc.scalar.copy(out=tk_sb, in_=tk_ps)

            # ---- y = Gh @ x  (natural [i, q] layout) ----
            y_sb = hbufs.tile([128, IC, p], F32R)
            for ic in range(IC):
                for qb in range(QB):
                    y_ps = ps_sc.tile([128, QW], F32, tag="sc", name="y")
                    for c in range(IC):
                        nc.tensor.matmul(
                            y_ps,
                            gt_sb[:, c, h, bass.ts(ic, 128)],
                            x_sb[:, c, bass.ts(qb, QW)],
                            start=(c == 0),
                            stop=(c == IC - 1),
                        )
                    if (ic * QB + qb) % 2 == 0:
                        nc.vector.tensor_copy(
                            out=y_sb[:, ic, bass.ts(qb, QW)], in_=y_ps
                        )
                    else:
                        nc.scalar.copy(
                            out=y_sb[:, ic, bass.ts(qb, QW)], in_=y_ps
                        )

            # ---- vT = (Wv_h x)^T  ([p, o] layout) ----
            # two p-chunks share one PSUM bank -> one wide evacuation
            vt_sb = hbufs.tile([128, KC, d], F32R)
            for pp in range(KC // 2):
                vt_ps = ps_sc.tile([128, 2, d], F32, tag="sc", name="vt")
                for sub in range(2):
                    pc = 2 * pp + sub
                    for c in range(IC):
                        nc.tensor.matmul(
                            vt_ps[:, sub, :],
                            x_sb[:, c, bass.ts(pc, 128)],
                            wv_sb[:, c, bass.ds(h * d, d)],
                            start=(c == 0),
                            stop=(c == IC - 1),
                        )
                if pp % 2 == 0:
                    nc.vector.tensor_copy(
                        out=vt_sb[:, bass.ts(pp, 2), :], in_=vt_ps
                    )
                else:
                    nc.scalar.copy(
                        out=vt_sb[:, bass.ts(pp, 2), :], in_=vt_ps
                    )

            for fqb in range(QB):
                if len(pending[fqb]) >= 2:
                    emit_proj(fqb, pending[fqb][:2], last_head=False)
                    pending[fqb] = pending[fqb][2:]

            for qb in range(QB):
                e_sb = ebufs.tile([128, KC, QW], F32R)
                s_ps = ps_s.tile([1, QW], F32)
                av_ps = [
                    ps_av.tile([128, QW], F32, tag="av", name=f"av{dc}")
                    for dc in range(IC)
                ]
                def emit_sums_av(kc):
                    # softmax denominator: s += 1^T E
                    nc.tensor.matmul(
                        s_ps,
                        ones_col,
                        e_sb[:, kc, :],
                        start=(kc == 0),
                        stop=(kc == KC - 1),
                    )
                    # AV accumulation
                    for dc in range(IC):
                        nc.tensor.matmul(
                            av_ps[dc],
                            vt_sb[:, kc, bass.ts(dc, 128)],
                            e_sb[:, kc, :],
                            start=(kc == 0),
                            stop=(kc == KC - 1),
                        )

                for kc in range(KC):
                    # attn chunk = x^T y + pos (identity-injected)
                    pos_t = pos_pool.tile([128, QW], F32R)
                    nc.sync.dma_start(
                        out=pos_t,
                        in_=pos_d[h, bass.ts(kc, 128), bass.ds(qb * QW, QW)],
                    )
                    at_ps = ps_at.tile([128, QW], F32)
                    for c in range(IC):
                        nc.tensor.matmul(
                            at_ps,
                            x_sb[:, c, bass.ts(kc, 128)],
                            y_sb[:, c, bass.ds(qb * QW, QW)],
                            start=(c == 0),
                            stop=False,
                        )
                    nc.tensor.matmul(
                        at_ps, ident, pos_t, start=False, stop=True
                    )
                    # E = exp(attn + term_k)
                    nc.scalar.activation(
                        out=e_sb[:, kc, :],
                        in_=at_ps,
                        func=mybir.ActivationFunctionType.Exp,
                        bias=tk_sb[:, kc : kc + 1],
                        scale=1.0,
                    )
                    # sums/av run one chunk behind so the PE never waits
                    # on the exp handoff
                    if kc > 0:
                        emit_sums_av(kc - 1)
                emit_sums_av(KC - 1)

                # normalization chain: recip (DVE) -> replicate (PE) ->
                # evac (ACT) -> normalize+evac out_h (DVE). Only the Wo
                # projection is deferred one block.
                r_sb = hbufs.tile([1, QW], F32)
                nc.vector.reciprocal(out=r_sb, in_=s_ps)
                repl_ps = ps_sc.tile([128, QW], F32, tag="sc", name="repl")
                nc.tensor.matmul(repl_ps, ones_row, r_sb, start=True, stop=True)
                rr_sb = hbufs.tile([128, QW], F32)
                nc.scalar.copy(out=rr_sb, in_=repl_ps)
                oh_sb = ohp.tile([128, IC, QW], F32R)
                for dc in range(IC):
                    nc.vector.tensor_mul(oh_sb[:, dc, :], av_ps[dc], rr_sb)

                pending[qb].append((h, oh_sb))
                if _rep == reps - 1 and h == h_num - 1:
                    # last head: flush this q-block's group now so the
                    # final projections overlap the remaining attention
                    emit_proj(qb, pending[qb], last_head=True)
                    pending[qb] = []

        for qb in range(QB):
            assert not pending[qb]

    nc.finalize()
    return nc


def prep_weights(Wk, bk, Wq, bq, Wv, bv, Wo, bo, h_num=H, d=D):
    """Host-side weight transformation (float64 accumulate)."""
    Wk = np.asarray(Wk, np.float64).reshape(h_num, d, d)
    Wq = np.asarray(Wq, np.float64).reshape(h_num, d, d)
    bq = np.asarray(bq, np.float64).reshape(h_num, d)
    Wv = np.asarray(Wv, np.float64)
    bv = np.asarray(bv, np.float64)
    Wo = np.asarray(Wo, np.float64)
    bo = np.asarray(bo, np.float64)
    s = 1.0 / np.sqrt(d)

    # lhsT for the y-matmul is Gh^T = Wq_h^T Wk_h * s
    gt = np.einsum("hdi,hdj->hij", Wq, Wk) * s
    u = np.einsum("hdi,hd->ih", Wk, bq) * s  # u[i, h]
    wv = Wv.T.copy()  # [d_in, h*d]
    wo = Wo.T.copy()  # [h*d, d_out]
    bo2 = bo + Wo @ bv
    return (
        gt.astype(np.float32),
        u.astype(np.float32),
        wv.astype(np.float32),
        wo.astype(np.float32),
        bo2.astype(np.float32),
    )


_NC_CACHE = {}


def _get_nc():
    if "nc" not in _NC_CACHE:
        _NC_CACHE["nc"] = build_nc()
    return _NC_CACHE["nc"]


def kernel(inputs, pos_mat, Wk, bk, Wq, bq, Wv, bv, Wo, bo, **run_kwargs):
    from concourse.bass_utils import run_bass_kernel_spmd

    x_all = np.ascontiguousarray(np.asarray(inputs, np.float32))  # [N, D, P]
    pos = np.ascontiguousarray(np.asarray(pos_mat, np.float32)[0])
    gt, u, wv, wo, bo2 = prep_weights(Wk, bk, Wq, bq, Wv, bv, Wo, bo)

    ident = np.eye(128, dtype=np.float32)
    onescol = np.ones((128, 1), np.float32)

    nc = _get_nc()
    in_maps = [
        {
            "x": x_all[n],
            "pos": pos,
            "gt": gt,
            "u": u,
            "wv": wv,
            "wo": wo,
            "bo": bo2,
            "ident": ident,
            "onescol": onescol,
        }
        for n in range(N)
    ]
    res = run_bass_kernel_spmd(nc, in_maps, core_ids=list(range(N)), **run_kwargs)
    out = np.stack([res.results[n]["out"] for n in range(N)])
    _NC_CACHE["last_result"] = res
    return out.astype(np.float32)
